# revision 57
# baseline (speedup 1.0000x reference)
"""Trainium2 Bass kernel for nn_Attention (dense transformer attention layer).

Reference semantics (bug-faithful to the source):
  - Q = x @ wq.T ; V = x @ wv.T ; K-projection is DEAD CODE (the reference
    overwrites xk with the double-angle-rotated Q, so wk never matters).
  - rot = double-angle RoPE applied to Q; keys == rot(Q).
  - start_pos == 0 and t == MAX_SEQ, so the KV cache contents never matter.
  - scores = rotQ @ rotQ.T / sqrt(HD) + mask ; P = softmax ; O = P @ V
  - out = O @ wo.T

Sharding (8 cores): core c -> batch b = c//2, head-half h = c%2 (8 of 16
heads).  Q/V projections + attention are (batch x head-half) parallel; each
core AllGathers its full-T per-head attention output within its pair, then
runs the output projection on its token-half with a full-D contraction.

Schedule (single TileContext, dataflow-overlapped):
  1. V projection for all tokens (x-stationary matmuls), releases wv.
  2. Q^T projection quarter 0, then per token-quarter n: attention chunk n
     head-interleaved with Q^T quarter n+1, so independent projection
     matmuls fill the PE queue wherever attention waits on the exp
     pipeline.  Q^T comes out of weight-stationary matmuls directly in
     [feat, tok] layout (no PE transposes); RoPE runs in that layout on
     r|i partition halves, enabled by a host-side deinterleaving column
     permutation of wq.
  3. Attention: scores in [k, q] layout (symmetric Gram matrix, causal
     wedge only), exp on the Scalar engine, PV + softmax-denominator
     (ones-vector) matmuls accumulating in PSUM; the causal mask is a
     post-exp 0/1 multiply on the diagonal blocks.  Per-(head, chunk)
     AllGather collectives (pairs) are staggered one head behind the
     normalize chain and stream throughout the kernel.
  4. Gather to o_full (dynamic chunk indices pick this core's token half)
     and the output projection over the full model dim.

All large inputs are host-cast to bf16 and host-tiled to match their SBUF
images exactly, so every DMA reads long contiguous runs per partition.
"""

import math
import sys

import numpy as np

sys.path.insert(0, "/opt/trn_rl_repo")

import concourse.bacc as bacc
import concourse.mybir as mybir
from concourse.tile import TileContext

F32 = mybir.dt.float32
BF16 = mybir.dt.bfloat16

B = 4
T = 2048
D = 2048
H = 16
HD = 128
N_CORES = 8
PAIRS = [[0, 1], [2, 3], [4, 5], [6, 7]]


def build_nc(T, D, H):
    HD = 128
    assert D == H * HD
    NH = H // 2          # heads per core (8)
    DQ = NH * HD         # own q/v feature count (1024)
    TH = T // 2          # token half
    NT = T // 128        # token tiles (16)
    ND = D // 128        # d tiles (16)
    NQ = 4               # token quarters
    XTQ = T // NQ        # tokens per quarter (512)
    QPC = XTQ // 128     # token tiles per quarter (4)
    CH = XTQ             # attention q-chunk width (512)
    scale = 1.0 / math.sqrt(HD)

    nc = bacc.Bacc(target_bir_lowering=False, num_devices=N_CORES)

    # inputs are host-tiled to match their SBUF images exactly, so every
    # DMA reads long contiguous runs per partition.  xt is token-tile
    # (j) major within each quarter so the first V-proj group depends on
    # only 0.5 MB; wvt is qc-major so the first group needs only half.
    xt = nc.declare_dram_parameter("xt", [NQ, 128, ND * XTQ], BF16, isOutput=False)
    wqt = nc.declare_dram_parameter("wqt", [128, ND * DQ], BF16, isOutput=False)
    wvt = nc.declare_dram_parameter("wvt", [128, ND * DQ], BF16, isOutput=False)
    # wo in [do, quarter, p, r16local*512+c] tiles for streamed o-proj
    woth = nc.declare_dram_parameter("woth", [4, 4, 128, 4 * 512], BF16,
                                     isOutput=False)
    mkt = nc.declare_dram_parameter("maskt", [128, 128], BF16, isOutput=False)
    # host-precomputed double-angle tables (partitions 0:64 cos2, 64:128 sin2)
    fcs = nc.declare_dram_parameter("fcs", [128, T], F32, isOutput=False)
    # out row-block r = 2*chunk + j holds this core's token tile j of chunk
    out = nc.declare_dram_parameter("out", [2 * NQ, 128, D], BF16, isOutput=True)

    # per-(head, chunk) pair exchange of attention output rows (chunk-major
    # so each chunk's collective operates on a contiguous block)
    ag_in = [nc.dram_tensor(f"agi{e}", [4, 128, T // 4], BF16) for e in range(NH)]
    ag_out = [nc.dram_tensor(f"ago{e}", [4, 2, 128, T // 4], BF16) for e in range(NH)]

    with TileContext(nc) as tc:
        import concourse.bass as bass_mod

        pid = nc.partition_id()
        h_idx = pid % 2
        peer_i = 1 - h_idx           # peer's index within the pair

        # ---------------- persistent / long-lived pools -----------------
        _cm = {}

        def popen(name, **kw):
            cm = tc.tile_pool(name=name, **kw)
            _cm[name] = cm
            return cm.__enter__()

        def pclose(name):
            _cm.pop(name).__exit__(None, None, None)

        p_c2s2 = popen("c2s2", bufs=1)
        p_rotqt = popen("rotqt", bufs=1)
        p_vsb = popen("vsb", bufs=1)
        p_wqt = popen("wqt", bufs=1, side="right")
        p_xt = popen("xt", bufs=2, side="right")
        p_misc = popen("misc", bufs=1)
        # persistent attention PSUM pools at the stack bottom; the top
        # 4-5 banks alternate between the projection pool (Q quarters)
        # and the 2-bank score mega-tiles (attention chunks)
        p_psO = popen("psO", bufs=2, space="PSUM")
        p_psD = popen("psD", bufs=1, space="PSUM")
        p_proj = popen("projps", bufs=2, space="PSUM")
        p_wvt = popen("wvt", bufs=1, side="right")

        rotqt = p_rotqt.tile([128, NH * T], BF16, tag="rotqt")
        v_sb = p_vsb.tile([128, NT * DQ], BF16, tag="vsb")
        # double-angle tables (partitions 0:64 c2, 64:128 s2, cols 0:T)
        c2s2 = p_c2s2.tile([128, T], F32, tag="c2s2")
        mkt_sb = p_misc.tile([128, 128], BF16, tag="mkt")
        ones_sb = p_misc.tile([128, 1], BF16, tag="ones")
        nc.vector.memset(ones_sb[:, :], 1.0)

        wvt_sb = p_wvt.tile([128, 2, ND, 512], BF16, tag="wvt")
        wqt_sb = p_wqt.tile([128, ND * DQ], BF16, tag="wqt")

        def load_wslices(dst_sb, src, engs, nsl=4):
            for q4 in range(nsl):
                w = ND * DQ // nsl
                cols = slice(q4 * w, (q4 + 1) * w)
                engs[q4 % len(engs)].dma_start(
                    out=dst_sb[:, cols], in_=src[:, cols]
                )

        def load_wvt(engs, nsl=8):
            # qc-major halves on separate rings, dk-ascending within each:
            # the first V groups are gated on only the qc=0 half, and qc=1
            # streams concurrently instead of queueing behind it
            flat = ND * 512
            ndk = ND // (nsl // 2)
            for qc in range(2):
                for q4 in range(nsl // 2):
                    w = flat // (nsl // 2)
                    engs[qc].dma_start(
                        out=wvt_sb[:, qc, q4 * ndk : (q4 + 1) * ndk, :],
                        in_=wvt[:, qc * flat + q4 * w : qc * flat + (q4 + 1) * w],
                    )

        def load_xt_quarter(n, engs, nsl=4):
            # j-major: each 128-token j block is a contiguous 2048-col run,
            # so the first V group is gated on a single 0.5 MB DMA
            xt_sb = p_xt.tile([128, QPC, ND, 128], BF16, tag="xtq")
            for j in range(QPC):
                engs[j % len(engs)].dma_start(
                    out=xt_sb[:, j, :, :],
                    in_=xt[n, :, j * ND * 128 : (j + 1) * ND * 128],
                )
            return xt_sb

        def prep_rope_tables():
            nc.scalar.dma_start(out=c2s2[:, :], in_=fcs[:, :])

        # ---------------- phase 1: V projection (all tokens) -------------
        # parallel queues in need-order: xt q0 j-blocks on sync, wvt
        # qc-halves on scalar, xt q1 on gpsimd — the first group starts
        # after ~1 MB of DMA instead of ~8 MB
        xt_pending = [load_xt_quarter(0, [nc.sync])]
        load_wvt([nc.scalar, nc.gpsimd])
        xt_pending.append(load_xt_quarter(1, [nc.sync]))
        for n in range(NQ):
            xt_sb = xt_pending.pop(0)
            if n < NQ - 1:
                xt_pending.append(
                    load_xt_quarter(
                        n + 2 if n < NQ - 2 else 0,
                        [nc.sync] if n % 2 else [nc.gpsimd],
                    )
                )
            if n == 0:
                nc.gpsimd.dma_start(out=mkt_sb[:, :], in_=mkt[:, :])
            if n == 1:
                # deferred loads: not needed until the Q phase
                load_wslices(wqt_sb, wqt, [nc.scalar, nc.gpsimd])
                prep_rope_tables()
            # qc outer on the first quarter so PE work starts after the
            # first wvt half lands
            for qc in range(2):
                for j in range(QPC):
                    tb = n * QPC + j
                    ps_v = p_proj.tile([128, 512], F32, tag="ps")
                    for dk in range(ND):
                        nc.tensor.matmul(
                            ps_v[:, :],
                            xt_sb[:, j, dk, :],
                            wvt_sb[:, qc, dk, :],
                            start=(dk == 0),
                            stop=(dk == ND - 1),
                        )
                    nc.vector.tensor_copy(
                        v_sb[:, tb * DQ + qc * 512 : tb * DQ + (qc + 1) * 512],
                        ps_v[:, :],
                    )
        pclose("wvt")

        # attention-phase pools (SBUF ring space freed by wvt)
        p_tt = popen("ttmp", bufs=2)
        p_pt = popen("pt", bufs=5)
        p_pts = popen("pts", bufs=3)
        p_rcp = popen("rcp", bufs=2)
        p_rcpb = popen("rcpb", bufs=3)
        p_otc = popen("otc", bufs=3)
        p_gsc = popen("gsc", bufs=3)
        p_ostg = popen("ostg", bufs=2)
        p_woth = popen("woth", bufs=4, side="right")
        p_osb = popen("osb", bufs=2)
        p_psS = popen("psS", bufs=3, space="PSUM")

        # ---------------- phase 2: Q^T + rope, interleaved attention ------
        def q_head(n, f, xt_sb):
            ps_q = p_proj.tile([128, 512], F32, tag="ps")
            for dk in range(ND):
                nc.tensor.matmul(
                    ps_q[:, :],
                    wqt_sb[:, dk * DQ + f * 128 : dk * DQ + (f + 1) * 128],
                    xt_sb[:, :, dk, :],
                    start=(dk == 0),
                    stop=(dk == ND - 1),
                )
            # rope in [feat, tok] layout: rows 0:64 real, 64:128 imag.
            # muls read PSUM+SBUF (mixed spaces, base-partition rule
            # exempt); the final sub/add reads two base-0 SBUF temps.
            qr = ps_q[0:64, :]
            qi = ps_q[64:128, :]
            c2n = c2s2[0:64, n * XTQ : (n + 1) * XTQ]
            s2n = c2s2[64:128, n * XTQ : (n + 1) * XTQ]
            col = slice(f * T + n * XTQ, f * T + (n + 1) * XTQ)
            t1a = p_tt.tile([64, 512], BF16, tag="a")
            t1b = p_tt.tile([64, 512], BF16, tag="b")
            nc.vector.tensor_mul(t1a[:, :], qr, c2n)
            nc.vector.tensor_mul(t1b[:, :], qi, s2n)
            nc.vector.tensor_sub(rotqt[0:64, col], t1a[:, :], t1b[:, :])
            t2a = p_tt.tile([64, 512], BF16, tag="c")
            t2b = p_tt.tile([64, 512], BF16, tag="d")
            nc.vector.tensor_mul(t2a[:, :], qr, s2n)
            nc.vector.tensor_mul(t2b[:, :], qi, c2n)
            nc.vector.tensor_add(rotqt[64:128, col], t2a[:, :], t2b[:, :])

        def emit_ag(eta, c):
            nc.gpsimd.collective_compute(
                "AllGather",
                mybir.AluOpType.bypass,
                replica_groups=PAIRS,
                ins=[ag_in[eta][c : c + 1, :, :].opt()],
                outs=[ag_out[eta][c : c + 1, :, :, :].opt()],
            )

        def attn_head(c, eta, fin=None):
            KC = (c + 1) * QPC
            q0 = c * CH
            ps_o = p_psO.tile([128, CH], F32, tag="pso")
            ps_d = p_psD.tile([1, CH], F32, tag="psd")
            # denominator: DVE sums exp-tile PAIRS, the PE reduces each
            # pair-sum with a ones-matmul into ps_d (half the DVE adds of
            # per-kt accumulation, half the PE cost of per-kt matmuls);
            # pend holds pair-sums whose matmul is staggered one pair back
            pend = []
            nmm = [0]

            def emit_den(last):
                pts, dqo = pend.pop(0)
                nc.tensor.matmul(
                    ps_d[:, dqo:CH],
                    ones_sb[:, :],
                    pts[:, dqo:CH],
                    start=(nmm[0] == 0),
                    stop=last,
                )
                nmm[0] += 1

            pt_prev = None
            for kt in range(KC):
                qo = max(0, (kt - c * QPC) * 128)
                ps_s = p_psS.tile([128, CH], F32, tag="pss")
                pt = p_pt.tile([128, CH], BF16, tag="pt")
                nc.tensor.matmul(
                    ps_s[:, qo:CH],
                    rotqt[:, eta * T + kt * 128 : eta * T + kt * 128 + 128],
                    rotqt[:, eta * T + q0 + qo : eta * T + q0 + CH],
                    start=True,
                    stop=True,
                )
                nc.scalar.activation(
                    pt[:, qo:CH],
                    ps_s[:, qo:CH],
                    mybir.ActivationFunctionType.Exp,
                    scale=scale,
                )
                if kt >= c * QPC:  # diagonal block: zero the causal part
                    nc.vector.tensor_mul(
                        pt[:, qo : qo + 128],
                        pt[:, qo : qo + 128],
                        mkt_sb[:, :],
                    )
                if kt % 2 == 1:
                    qo0 = max(0, (kt - 1 - c * QPC) * 128)
                    pts = p_pts.tile([128, CH], BF16, tag="pts")
                    if qo > qo0:
                        nc.vector.tensor_copy(
                            pts[:, qo0:qo], pt_prev[:, qo0:qo]
                        )
                    nc.vector.tensor_add(
                        pts[:, qo:CH], pt_prev[:, qo:CH], pt[:, qo:CH]
                    )
                    pend.append((pts, qo0))
                    if len(pend) > 1:
                        emit_den(False)
                nc.tensor.matmul(
                    ps_o[:, qo:CH],
                    v_sb[:, kt * DQ + eta * 128 : kt * DQ + eta * 128 + 128],
                    pt[:, qo:CH],
                    start=(kt == 0),
                    stop=(kt == KC - 1),
                )
                pt_prev = pt
                if kt == 0 and fin is not None:
                    fin()  # older heads' normalize chains, staggered
            return {"c": c, "eta": eta, "ps_o": ps_o, "ps_d": ps_d,
                    "pend": pend, "emit_den": emit_den}

        def fin_a(tk):
            # one head behind: denominator flush + reciprocal + broadcast
            # kickoff + PSUM drain.  Nothing here waits on gpsimd.
            pend, emit_den = tk["pend"], tk["emit_den"]
            while pend:
                emit_den(not pend[1:])
            rcp = p_rcp.tile([1, CH], F32, tag="rcp")
            rcph = p_rcp.tile([1, CH], BF16, tag="rcph")
            rcpb = p_rcpb.tile([128, CH], BF16, tag="rcpb")
            nc.vector.reciprocal_approx_fast(rcp[:, :], tk["ps_d"][:, :])
            nc.vector.tensor_copy(rcph[:, :], rcp[:, :])
            ocp = p_otc.tile([128, CH], BF16, tag="ocp")
            nc.vector.tensor_copy(ocp[:, :], tk["ps_o"][:, :])
            nc.gpsimd.partition_broadcast(rcpb[:, :], rcph[:, :])
            tk["ocp"], tk["rcpb"] = ocp, rcpb

        # per-chunk o-proj staging: own rows land via SBUF->SBUF copy from
        # otc; peer rows stream from ag_out one head-step after each AG
        ostg = {}
        ag_fifo = []

        def get_ostg(c):
            if c not in ostg:
                ostg[c] = p_ostg.tile(
                    [128, 2 * NH * 256], BF16, tag="ostg", name=f"ostg{c}"
                )
            return ostg[c]

        def drain_peer_gathers(keep):
            # two-hop gather: hop 1 has the dynamic (peer_i, token-half)
            # source — ds() APs are invisible to the dependency tracker, so
            # its write lands in a scratch tile whose only hazard is
            # far-apart pool reuse; hop 2 is fully static (same ring, FIFO
            # ordered after hop 1) so consumers get a tracked dependency.
            while len(ag_fifo) > keep:
                eta, c = ag_fifo.pop(0)
                gsc = p_gsc.tile([128, 256], BF16, tag="gsc",
                                 name=f"gsc{eta}_{c}")
                nc.gpsimd.dma_start(
                    out=gsc[:, :],
                    in_=ag_out[eta][
                        c, bass_mod.ds(peer_i, 1), :,
                        bass_mod.ds(h_idx * 256, 256)
                    ],
                )
                nc.gpsimd.dma_start(
                    out=get_ostg(c)[:, (NH + eta) * 256 : (NH + eta + 1) * 256],
                    in_=gsc[:, :],
                )

        def fin_b(tk, drain=True):
            # two heads behind: by now the broadcast is long done, so the
            # DVE normalize never stalls the queue
            c, eta = tk["c"], tk["eta"]
            otc = p_otc.tile([128, CH], BF16, tag="otc")
            nc.vector.tensor_mul(otc[:, :], tk["ocp"][:, :], tk["rcpb"][:, :])
            nc.sync.dma_start(out=ag_in[eta][c, :, :], in_=otc[:, :])
            # own token-half, two-hop for the same tracked-dep reason
            osc = p_gsc.tile([128, 256], BF16, tag="osc",
                             name=f"osc{eta}_{c}")
            nc.sync.dma_start(
                out=osc[:, :], in_=otc[:, bass_mod.ds(h_idx * 256, 256)]
            )
            nc.sync.dma_start(
                out=get_ostg(c)[:, eta * 256 : (eta + 1) * 256],
                in_=osc[:, :],
            )
            if eta >= 1:
                emit_ag(eta - 1, c)
                ag_fifo.append((eta - 1, c))
            if eta == NH - 1:
                emit_ag(NH - 1, c)
                ag_fifo.append((NH - 1, c))
            if drain:
                drain_peer_gathers(0)

        # ---- interleaved output projection ------------------------------
        # wot streams as [do, quarter] tiles of 4 r16-rows x 512 cols; one
        # (c, do) pair of token tiles consumes quarters 4*do..4*do+3
        woth_order = [(do, q) for _ in range(NQ) for do in range(4)
                      for q in range(4)]
        wq_ptr = [0]
        woth_eng = [nc.gpsimd]

        def issue_woth(nload):
            for _ in range(nload):
                if wq_ptr[0] >= len(woth_order):
                    return
                do, q = woth_order[wq_ptr[0]]
                wq_ptr[0] += 1
                wt = p_woth.tile([128, 4 * 512], BF16, tag="woth")
                woth_eng[0].dma_start(out=wt[:, :], in_=woth[do, q, :, :])
                woth_tiles.append(wt)

        woth_tiles = []

        def oproj_tile(c, t):
            do, j = t // 2, t % 2
            g = c * 8 + t
            if j == 1:
                issue_woth(4)  # next do-pair streams while this one runs
            stg = get_ostg(c)
            ps_out = p_proj.tile([128, 512], F32, tag="ps")
            base = (g // 2) * 4
            for r16 in range(2 * NH):
                wt = woth_tiles[base + r16 // 4]
                nc.tensor.matmul(
                    ps_out[:, :],
                    stg[:, r16 * 256 + j * 128 : r16 * 256 + j * 128 + 128],
                    wt[:, (r16 % 4) * 512 : (r16 % 4 + 1) * 512],
                    start=(r16 == 0),
                    stop=(r16 == 2 * NH - 1),
                )
            osb = p_osb.tile([128, 512], BF16, tag="osb")
            nc.vector.tensor_copy(osb[:, :], ps_out[:, :])
            nc.sync.dma_start(
                out=out[2 * c + j, :, do * 512 : (do + 1) * 512],
                in_=osb[:, :],
            )
            if t == 7:
                ostg.pop(c, None)

        # Q(0) stands alone; thereafter Q(n+1) head f interleaves with
        # attention chunk n head f, so independent projection matmuls fill
        # the PE queue wherever attention stalls on the exp pipeline.
        xtq = xt_pending.pop(0)
        for f in range(NH):
            q_head(0, f, xtq)
        prev1 = prev2 = None  # prev1 awaits fin_a, prev2 awaits fin_b
        for n in range(NQ):
            xt_prev, xtq = xtq, None
            if n + 1 < NQ:
                xtq = load_xt_quarter(n + 1, [nc.sync])
            if n == 1:
                issue_woth(2)
            for eta in range(NH):
                def fin(p1=prev1, p2=prev2):
                    if p1 is not None:
                        fin_a(p1)
                    if p2 is not None:
                        fin_b(p2)
                prev2, prev1 = prev1, attn_head(n, eta, fin)
                if n + 1 < NQ:
                    q_head(n + 1, eta, xtq)
                # o-proj tiles, three head-steps behind the chunk's AGs so
                # every peer gather lands at least one step before its
                # first consumer
                if n == 1 and eta == 1:
                    issue_woth(2)
                if eta >= 3 and n >= 1:
                    oproj_tile(n - 1, eta - 3)
                elif eta < 3 and n >= 2:
                    oproj_tile(n - 2, 5 + eta)
        # tail: flush fins with no gather drains between the final AllToAll
        # triggers, fill the collective flight time with chunk-2 tiles,
        # then drain and run chunk 3.  woth loads move to the sync ring so
        # they never queue behind AG-waiting gathers on gpsimd.
        woth_eng[0] = nc.sync
        fin_a(prev1)
        fin_b(prev2, drain=False)
        fin_b(prev1, drain=False)
        for t in (5, 6, 7):
            oproj_tile(2, t)
        drain_peer_gathers(0)
        for t in range(8):
            oproj_tile(3, t)

        for name in reversed(list(_cm)):
            pclose(name)

    nc.finalize()
    return nc


def host_prep(T, D, H, x, wq, wv, wo, mask, freqs_cos, freqs_sin):
    """Build per-core in_maps (host-side layout/dtype prep only)."""
    import ml_dtypes

    bf16 = ml_dtypes.bfloat16
    HD = 128
    NH = H // 2
    DQ = NH * HD
    # 0/1 keep-mask (from the additive mask) for post-exp diagonal zeroing
    m128 = np.asarray(mask, np.float32).reshape(T, T)[:128, :128]
    mkt = np.ascontiguousarray(np.where(m128 < -1e8, 0.0, 1.0).astype(bf16))
    fcn = np.asarray(freqs_cos, np.float32)  # [T, 64]
    fsn = np.asarray(freqs_sin, np.float32)
    c2t = fcn.T * fcn.T - fsn.T * fsn.T   # cos(2a)  [64, T]
    s2t = 2.0 * fcn.T * fsn.T             # sin(2a)
    # partitions 0:64 cos2, 64:128 sin2
    fcs = np.ascontiguousarray(np.concatenate([c2t, s2t], axis=0).astype(np.float32))
    # deinterleave permutation: within each head block, (r0,r1,..,i0,i1,..)
    perm = np.concatenate(
        [hb * 128 + np.r_[0:128:2, 1:128:2] for hb in range(NH)]
    )
    wot_full = np.asarray(wo, np.float32).T  # [din2, dout]
    in_maps = []
    for c in range(N_CORES):
        b, h = c // 2, c % 2
        rows = slice(h * DQ, (h + 1) * DQ)
        wqt_c = np.asarray(wq[rows], np.float32).T[:, perm]
        # stg rows are own-heads-first: permute wot accordingly
        wot_c = np.concatenate(
            [wot_full[h * DQ : (h + 1) * DQ], wot_full[(1 - h) * DQ : (2 - h) * DQ]],
            axis=0,
        )
        # tile to SBUF images: [dk-major columns, partition-major rows]
        def timg(a, ncol):  # [D, C] -> [C//ncol, 128, 16*ncol]
            ND_, C = a.shape[0] // 128, a.shape[1]
            return np.ascontiguousarray(
                a.reshape(ND_, 128, C // ncol, ncol)
                .transpose(2, 1, 0, 3)
                .reshape(C // ncol, 128, ND_ * ncol)
                .astype(bf16)
            )
        # xt: j-major within each quarter: [n, p, j, dk, t]
        xT = np.asarray(x[b], np.float32).T                      # [D, T]
        xtc = np.ascontiguousarray(
            xT.reshape(16, 128, 4, 4, 128)                       # dk,p,n,j,t
            .transpose(2, 1, 3, 0, 4)
            .reshape(4, 128, 4 * 16 * 128)
            .astype(bf16)
        )
        wqtc = timg(wqt_c, DQ).reshape(128, -1)                  # [128,16384]
        # wvt: qc-major: [p, qc, dk, c]
        wvT = np.asarray(wv[rows], np.float32).T                 # [D, DQ]
        wvtc = np.ascontiguousarray(
            wvT.reshape(16, 128, 2, 512)                         # dk,p,qc,c
            .transpose(1, 2, 0, 3)
            .reshape(128, 2 * 16 * 512)
            .astype(bf16)
        )
        # woth: [do, quarter, p, r16local, c]; rows own-heads-first
        wothc = np.ascontiguousarray(
            wot_c.reshape(4, 4, 128, 4, 512)     # q, r16l, p, do, c
            .transpose(3, 0, 2, 1, 4)
            .reshape(4, 4, 128, 4 * 512)
            .astype(bf16)
        )
        in_maps.append(
            {
                "xt": xtc,
                "wqt": wqtc,
                "wvt": wvtc,
                "woth": wothc,
                "maskt": mkt,
                "fcs": fcs,
            }
        )
    return in_maps


_NC_CACHE = {}


def run(T, D, H, inputs, trace=False):
    from concourse.bass_utils import run_bass_kernel_spmd

    key = (T, D, H)
    if key not in _NC_CACHE:
        _NC_CACHE[key] = build_nc(T, D, H)
    nc = _NC_CACHE[key]
    in_maps = host_prep(
        T, D, H,
        inputs["x"], inputs["wq"], inputs["wv"], inputs["wo"],
        inputs["mask"], inputs["freqs_cos"], inputs["freqs_sin"],
    )
    res = run_bass_kernel_spmd(nc, in_maps, core_ids=list(range(N_CORES)), trace=trace)
    B_ = np.asarray(inputs["x"]).shape[0]
    out = np.empty((B_, T, D), np.float32)
    for c in range(N_CORES):
        b, h = c // 2, c % 2
        oc = np.asarray(res.results[c]["out"], np.float32)  # [8,128,D]
        for ck in range(4):
            for j in range(2):
                t0 = (4 * ck + 2 * h + j) * 128
                out[b, t0 : t0 + 128, :] = oc[2 * ck + j]
    return out, res


def kernel(**inputs):
    out, _ = run(T, D, H, inputs, trace=False)
    return out



# revision 59
# speedup vs baseline: 1.1580x; 1.1580x over previous
"""Trainium2 Bass kernel for nn_Attention (dense transformer attention layer).

Reference semantics (bug-faithful to the source):
  - Q = x @ wq.T ; V = x @ wv.T ; K-projection is DEAD CODE (the reference
    overwrites xk with the double-angle-rotated Q, so wk never matters).
  - rot = double-angle RoPE applied to Q; keys == rot(Q).
  - start_pos == 0 and t == MAX_SEQ, so the KV cache contents never matter.
  - scores = rotQ @ rotQ.T / sqrt(HD) + mask ; P = softmax ; O = P @ V
  - out = O @ wo.T

Sharding (8 cores): core c -> batch b = c//2, head-half h = c%2 (8 of 16
heads).  Q/V projections + attention are (batch x head-half) parallel; each
core AllGathers its full-T per-head attention output within its pair, then
runs the output projection on its token-half with a full-D contraction.

Schedule (single TileContext, dataflow-overlapped):
  1. V projection for all tokens (x-stationary matmuls), releases wv.
  2. Q^T projection quarter 0, then per token-quarter n: attention chunk n
     head-interleaved with Q^T quarter n+1, so independent projection
     matmuls fill the PE queue wherever attention waits on the exp
     pipeline.  Q^T comes out of weight-stationary matmuls directly in
     [feat, tok] layout (no PE transposes); RoPE runs in that layout on
     r|i partition halves, enabled by a host-side deinterleaving column
     permutation of wq.
  3. Attention: scores in [k, q] layout (symmetric Gram matrix, causal
     wedge only), exp on the Scalar engine, PV + softmax-denominator
     (ones-vector) matmuls accumulating in PSUM; the causal mask is a
     post-exp 0/1 multiply on the diagonal blocks.  Per-(head, chunk)
     AllGather collectives (pairs) are staggered one head behind the
     normalize chain and stream throughout the kernel.
  4. Gather to o_full (dynamic chunk indices pick this core's token half)
     and the output projection over the full model dim.

All large inputs are host-cast to bf16 and host-tiled to match their SBUF
images exactly, so every DMA reads long contiguous runs per partition.
"""

import math
import sys

import numpy as np

sys.path.insert(0, "/opt/trn_rl_repo")

import concourse.bacc as bacc
import concourse.mybir as mybir
from concourse.tile import TileContext

F32 = mybir.dt.float32
BF16 = mybir.dt.bfloat16

B = 4
T = 2048
D = 2048
H = 16
HD = 128
N_CORES = 8
PAIRS = [[0, 1], [2, 3], [4, 5], [6, 7]]


def build_nc(T, D, H):
    HD = 128
    assert D == H * HD
    NH = H // 2          # heads per core (8)
    DQ = NH * HD         # own q/v feature count (1024)
    TH = T // 2          # token half
    NT = T // 128        # token tiles (16)
    ND = D // 128        # d tiles (16)
    NQ = 4               # token quarters
    XTQ = T // NQ        # tokens per quarter (512)
    QPC = XTQ // 128     # token tiles per quarter (4)
    CH = XTQ             # attention q-chunk width (512)
    scale = 1.0 / math.sqrt(HD)

    nc = bacc.Bacc(target_bir_lowering=False, num_devices=N_CORES)

    # inputs are host-tiled to match their SBUF images exactly, so every
    # DMA reads long contiguous runs per partition.  xt is token-tile
    # (j) major within each quarter so the first V-proj group depends on
    # only 0.5 MB; wvt is qc-major so the first group needs only half.
    xt = nc.declare_dram_parameter("xt", [NQ, 128, ND * XTQ], BF16, isOutput=False)
    wqt = nc.declare_dram_parameter("wqt", [128, ND * DQ], BF16, isOutput=False)
    wvt = nc.declare_dram_parameter("wvt", [128, ND * DQ], BF16, isOutput=False)
    # wo in [do, quarter, p, r16local*512+c] tiles for streamed o-proj
    woth = nc.declare_dram_parameter("woth", [4, 4, 128, 4 * 512], BF16,
                                     isOutput=False)
    mkt = nc.declare_dram_parameter("maskt", [128, 128], BF16, isOutput=False)
    # host-precomputed double-angle tables (partitions 0:64 cos2, 64:128 sin2)
    fcs = nc.declare_dram_parameter("fcs", [128, T], F32, isOutput=False)
    # out row-block r = 2*chunk + j holds this core's token tile j of chunk
    out = nc.declare_dram_parameter("out", [2 * NQ, 128, D], BF16, isOutput=True)

    # per-(head, chunk) pair exchange of attention output rows (chunk-major
    # so each chunk's collective operates on a contiguous block)
    ag_in = [nc.dram_tensor(f"agi{e}", [4, 128, T // 4], BF16) for e in range(NH)]
    ag_out = [nc.dram_tensor(f"ago{e}", [4, 2, 128, T // 4], BF16) for e in range(NH)]

    with TileContext(nc) as tc:
        import concourse.bass as bass_mod

        pid = nc.partition_id()
        h_idx = pid % 2
        peer_i = 1 - h_idx           # peer's index within the pair

        # ---------------- persistent / long-lived pools -----------------
        _cm = {}

        def popen(name, **kw):
            cm = tc.tile_pool(name=name, **kw)
            _cm[name] = cm
            return cm.__enter__()

        def pclose(name):
            _cm.pop(name).__exit__(None, None, None)

        p_c2s2 = popen("c2s2", bufs=1)
        p_rotqt = popen("rotqt", bufs=1)
        p_vsb = popen("vsb", bufs=1)
        p_wqt = popen("wqt", bufs=1, side="right")
        p_xt = popen("xt", bufs=2, side="right")
        p_misc = popen("misc", bufs=1)
        # persistent attention PSUM pools at the stack bottom; the top
        # 4-5 banks alternate between the projection pool (Q quarters)
        # and the 2-bank score mega-tiles (attention chunks)
        p_psO = popen("psO", bufs=2, space="PSUM")
        p_psD = popen("psD", bufs=1, space="PSUM")
        p_proj = popen("projps", bufs=2, space="PSUM")
        p_wvt = popen("wvt", bufs=1, side="right")

        rotqt = p_rotqt.tile([128, NH * T], BF16, tag="rotqt")
        v_sb = p_vsb.tile([128, NT * DQ], BF16, tag="vsb")
        # double-angle tables (partitions 0:64 c2, 64:128 s2, cols 0:T)
        c2s2 = p_c2s2.tile([128, T], F32, tag="c2s2")
        mkt_sb = p_misc.tile([128, 128], BF16, tag="mkt")
        ones_sb = p_misc.tile([128, 1], BF16, tag="ones")
        nc.vector.memset(ones_sb[:, :], 1.0)

        wvt_sb = p_wvt.tile([128, 2, ND, 512], BF16, tag="wvt")
        wqt_sb = p_wqt.tile([128, ND * DQ], BF16, tag="wqt")

        def load_wslices(dst_sb, src, engs, nsl=4):
            for q4 in range(nsl):
                w = ND * DQ // nsl
                cols = slice(q4 * w, (q4 + 1) * w)
                engs[q4 % len(engs)].dma_start(
                    out=dst_sb[:, cols], in_=src[:, cols]
                )

        def load_wvt(engs, nsl=8):
            # qc-major halves on separate rings, dk-ascending within each:
            # the first V groups are gated on only the qc=0 half, and qc=1
            # streams concurrently instead of queueing behind it
            flat = ND * 512
            ndk = ND // (nsl // 2)
            for qc in range(2):
                for q4 in range(nsl // 2):
                    w = flat // (nsl // 2)
                    engs[qc].dma_start(
                        out=wvt_sb[:, qc, q4 * ndk : (q4 + 1) * ndk, :],
                        in_=wvt[:, qc * flat + q4 * w : qc * flat + (q4 + 1) * w],
                    )

        def load_xt_quarter(n, engs, nsl=4):
            # j-major: each 128-token j block is a contiguous 2048-col run,
            # so the first V group is gated on a single 0.5 MB DMA
            xt_sb = p_xt.tile([128, QPC, ND, 128], BF16, tag="xtq")
            for j in range(QPC):
                engs[j % len(engs)].dma_start(
                    out=xt_sb[:, j, :, :],
                    in_=xt[n, :, j * ND * 128 : (j + 1) * ND * 128],
                )
            return xt_sb

        def prep_rope_tables():
            nc.scalar.dma_start(out=c2s2[:, :], in_=fcs[:, :])

        # ---------------- phase 1: V projection (all tokens) -------------
        # parallel queues in need-order: xt q0 j-blocks on sync, wvt
        # qc-halves on scalar, xt q1 on gpsimd — the first group starts
        # after ~1 MB of DMA instead of ~8 MB
        xt_pending = [load_xt_quarter(0, [nc.sync])]
        load_wvt([nc.scalar, nc.gpsimd])
        xt_pending.append(load_xt_quarter(1, [nc.sync]))
        for n in range(NQ):
            xt_sb = xt_pending.pop(0)
            if n < NQ - 1:
                xt_pending.append(
                    load_xt_quarter(
                        n + 2 if n < NQ - 2 else 0,
                        [nc.sync] if n % 2 else [nc.gpsimd],
                    )
                )
            if n == 0:
                nc.gpsimd.dma_start(out=mkt_sb[:, :], in_=mkt[:, :])
            if n == 1:
                # deferred loads: not needed until the Q phase
                load_wslices(wqt_sb, wqt, [nc.scalar, nc.gpsimd])
                prep_rope_tables()
            # qc outer on the first quarter so PE work starts after the
            # first wvt half lands
            for qc in range(2):
                for j in range(QPC):
                    tb = n * QPC + j
                    ps_v = p_proj.tile([128, 512], F32, tag="ps")
                    for dk in range(ND):
                        nc.tensor.matmul(
                            ps_v[:, :],
                            xt_sb[:, j, dk, :],
                            wvt_sb[:, qc, dk, :],
                            start=(dk == 0),
                            stop=(dk == ND - 1),
                        )
                    nc.vector.tensor_copy(
                        v_sb[:, tb * DQ + qc * 512 : tb * DQ + (qc + 1) * 512],
                        ps_v[:, :],
                    )
        pclose("wvt")

        # attention-phase pools (SBUF ring space freed by wvt)
        p_tt = popen("ttmp", bufs=2)
        p_pt = popen("pt", bufs=5)
        p_pts = popen("pts", bufs=3)
        p_rcp = popen("rcp", bufs=2)
        p_rcpb = popen("rcpb", bufs=3)
        p_otc = popen("otc", bufs=3)
        p_gsc = popen("gsc", bufs=3)
        p_ostg = popen("ostg", bufs=2)
        p_woth = popen("woth", bufs=4, side="right")
        p_osb = popen("osb", bufs=2)
        p_psS = popen("psS", bufs=3, space="PSUM")

        # ---------------- phase 2: Q^T + rope, interleaved attention ------
        def q_head(n, f, xt_sb):
            ps_q = p_proj.tile([128, 512], F32, tag="ps")
            for dk in range(ND):
                nc.tensor.matmul(
                    ps_q[:, :],
                    wqt_sb[:, dk * DQ + f * 128 : dk * DQ + (f + 1) * 128],
                    xt_sb[:, :, dk, :],
                    start=(dk == 0),
                    stop=(dk == ND - 1),
                )
            # rope in [feat, tok] layout: rows 0:64 real, 64:128 imag.
            # muls read PSUM+SBUF (mixed spaces, base-partition rule
            # exempt); the final sub/add reads two base-0 SBUF temps.
            qr = ps_q[0:64, :]
            qi = ps_q[64:128, :]
            c2n = c2s2[0:64, n * XTQ : (n + 1) * XTQ]
            s2n = c2s2[64:128, n * XTQ : (n + 1) * XTQ]
            col = slice(f * T + n * XTQ, f * T + (n + 1) * XTQ)
            t1a = p_tt.tile([64, 512], BF16, tag="a")
            t1b = p_tt.tile([64, 512], BF16, tag="b")
            nc.vector.tensor_mul(t1a[:, :], qr, c2n)
            nc.vector.tensor_mul(t1b[:, :], qi, s2n)
            nc.vector.tensor_sub(rotqt[0:64, col], t1a[:, :], t1b[:, :])
            t2a = p_tt.tile([64, 512], BF16, tag="c")
            t2b = p_tt.tile([64, 512], BF16, tag="d")
            nc.vector.tensor_mul(t2a[:, :], qr, s2n)
            nc.vector.tensor_mul(t2b[:, :], qi, c2n)
            nc.vector.tensor_add(rotqt[64:128, col], t2a[:, :], t2b[:, :])

        def emit_ag(eta, c):
            nc.gpsimd.collective_compute(
                "AllGather",
                mybir.AluOpType.bypass,
                replica_groups=PAIRS,
                ins=[ag_in[eta][c : c + 1, :, :].opt()],
                outs=[ag_out[eta][c : c + 1, :, :, :].opt()],
            )

        def attn_head(c, eta, fin=None):
            KC = (c + 1) * QPC
            q0 = c * CH
            ps_o = p_psO.tile([128, CH], F32, tag="pso")
            ps_d = p_psD.tile([1, CH], F32, tag="psd")
            # denominator: DVE sums exp-tile PAIRS, the PE reduces each
            # pair-sum with a ones-matmul into ps_d (half the DVE adds of
            # per-kt accumulation, half the PE cost of per-kt matmuls);
            # pend holds pair-sums whose matmul is staggered one pair back
            pend = []
            nmm = [0]

            def emit_den(last):
                pts, dqo = pend.pop(0)
                nc.tensor.matmul(
                    ps_d[:, dqo:CH],
                    ones_sb[:, :],
                    pts[:, dqo:CH],
                    start=(nmm[0] == 0),
                    stop=last,
                )
                nmm[0] += 1

            pt_prev = None
            for kt in range(KC):
                qo = max(0, (kt - c * QPC) * 128)
                ps_s = p_psS.tile([128, CH], F32, tag="pss")
                pt = p_pt.tile([128, CH], BF16, tag="pt")
                nc.tensor.matmul(
                    ps_s[:, qo:CH],
                    rotqt[:, eta * T + kt * 128 : eta * T + kt * 128 + 128],
                    rotqt[:, eta * T + q0 + qo : eta * T + q0 + CH],
                    start=True,
                    stop=True,
                )
                nc.scalar.activation(
                    pt[:, qo:CH],
                    ps_s[:, qo:CH],
                    mybir.ActivationFunctionType.Exp,
                    scale=scale,
                )
                if kt >= c * QPC:  # diagonal block: zero the causal part
                    nc.vector.tensor_mul(
                        pt[:, qo : qo + 128],
                        pt[:, qo : qo + 128],
                        mkt_sb[:, :],
                    )
                if kt % 2 == 1:
                    qo0 = max(0, (kt - 1 - c * QPC) * 128)
                    pts = p_pts.tile([128, CH], BF16, tag="pts")
                    if qo > qo0:
                        nc.vector.tensor_copy(
                            pts[:, qo0:qo], pt_prev[:, qo0:qo]
                        )
                    nc.vector.tensor_add(
                        pts[:, qo:CH], pt_prev[:, qo:CH], pt[:, qo:CH]
                    )
                    pend.append((pts, qo0))
                    if len(pend) > 1:
                        emit_den(False)
                nc.tensor.matmul(
                    ps_o[:, qo:CH],
                    v_sb[:, kt * DQ + eta * 128 : kt * DQ + eta * 128 + 128],
                    pt[:, qo:CH],
                    start=(kt == 0),
                    stop=(kt == KC - 1),
                )
                pt_prev = pt
                if kt == 0 and fin is not None:
                    fin()  # older heads' normalize chains, staggered
            return {"c": c, "eta": eta, "ps_o": ps_o, "ps_d": ps_d,
                    "pend": pend, "emit_den": emit_den}

        def fin_a(tk):
            # one head behind: denominator flush + reciprocal + broadcast
            # kickoff + PSUM drain.  Nothing here waits on gpsimd.
            pend, emit_den = tk["pend"], tk["emit_den"]
            while pend:
                emit_den(not pend[1:])
            rcp = p_rcp.tile([1, CH], F32, tag="rcp")
            rcph = p_rcp.tile([1, CH], BF16, tag="rcph")
            rcpb = p_rcpb.tile([128, CH], BF16, tag="rcpb")
            nc.vector.reciprocal_approx_fast(rcp[:, :], tk["ps_d"][:, :])
            nc.vector.tensor_copy(rcph[:, :], rcp[:, :])
            ocp = p_otc.tile([128, CH], BF16, tag="ocp")
            nc.vector.tensor_copy(ocp[:, :], tk["ps_o"][:, :])
            nc.gpsimd.partition_broadcast(rcpb[:, :], rcph[:, :])
            tk["ocp"], tk["rcpb"] = ocp, rcpb

        # per-chunk o-proj staging: own rows land via SBUF->SBUF copy from
        # otc; peer rows stream from ag_out one head-step after each AG
        ostg = {}
        ag_fifo = []

        def get_ostg(c):
            if c not in ostg:
                ostg[c] = p_ostg.tile(
                    [128, 2 * NH * 256], BF16, tag="ostg", name=f"ostg{c}"
                )
            return ostg[c]

        agdum = p_misc.tile([2, 16], BF16, tag="agdum")

        def drain_peer_gathers(keep):
            # two-hop gathers for BOTH halves out of the AllGather result.
            # The agdum read is static, so it carries the tracked
            # AG-complete dependency and ring FIFO orders the dynamic hop-1
            # reads behind it; hop 2 is fully static so the o-proj matmuls
            # get a tracked dependency on the staging writes.
            while len(ag_fifo) > keep:
                eta, c = ag_fifo.pop(0)
                nc.gpsimd.dma_start(
                    out=agdum[:, :], in_=ag_out[eta][c, :, 0:1, 0:16]
                )
                gsc = p_gsc.tile([128, 256], BF16, tag="gsc",
                                 name=f"gsc{eta}_{c}")
                osc = p_gsc.tile([128, 256], BF16, tag="osc",
                                 name=f"osc{eta}_{c}")
                nc.gpsimd.dma_start(
                    out=gsc[:, :],
                    in_=ag_out[eta][
                        c, bass_mod.ds(peer_i, 1), :,
                        bass_mod.ds(h_idx * 256, 256)
                    ],
                )
                nc.gpsimd.dma_start(
                    out=osc[:, :],
                    in_=ag_out[eta][
                        c, bass_mod.ds(h_idx, 1), :,
                        bass_mod.ds(h_idx * 256, 256)
                    ],
                )
                nc.gpsimd.dma_start(
                    out=get_ostg(c)[:, (NH + eta) * 256 : (NH + eta + 1) * 256],
                    in_=gsc[:, :],
                )
                nc.gpsimd.dma_start(
                    out=get_ostg(c)[:, eta * 256 : (eta + 1) * 256],
                    in_=osc[:, :],
                )

        def fin_b(tk, drain=True):
            # two heads behind: by now the broadcast is long done, so the
            # DVE normalize never stalls the queue
            c, eta = tk["c"], tk["eta"]
            otc = p_otc.tile([128, CH], BF16, tag="otc")
            nc.vector.tensor_mul(otc[:, :], tk["ocp"][:, :], tk["rcpb"][:, :])
            nc.sync.dma_start(out=ag_in[eta][c, :, :], in_=otc[:, :])
            if eta >= 1:
                emit_ag(eta - 1, c)
                ag_fifo.append((eta - 1, c))
            if eta == NH - 1:
                emit_ag(NH - 1, c)
                ag_fifo.append((NH - 1, c))
            if drain:
                drain_peer_gathers(0)

        # ---- interleaved output projection ------------------------------
        # wot streams as [do, quarter] tiles of 4 r16-rows x 512 cols; one
        # (c, do) pair of token tiles consumes quarters 4*do..4*do+3
        woth_order = [(do, q) for _ in range(NQ) for do in range(4)
                      for q in range(4)]
        wq_ptr = [0]
        woth_eng = [nc.gpsimd]

        def issue_woth(nload):
            for _ in range(nload):
                if wq_ptr[0] >= len(woth_order):
                    return
                do, q = woth_order[wq_ptr[0]]
                wq_ptr[0] += 1
                wt = p_woth.tile([128, 4 * 512], BF16, tag="woth")
                woth_eng[0].dma_start(out=wt[:, :], in_=woth[do, q, :, :])
                woth_tiles.append(wt)

        woth_tiles = []

        def oproj_tile(c, t):
            do, j = t // 2, t % 2
            g = c * 8 + t
            if j == 1:
                issue_woth(4)  # next do-pair streams while this one runs
            stg = get_ostg(c)
            ps_out = p_proj.tile([128, 512], F32, tag="ps")
            base = (g // 2) * 4
            for r16 in range(2 * NH):
                wt = woth_tiles[base + r16 // 4]
                nc.tensor.matmul(
                    ps_out[:, :],
                    stg[:, r16 * 256 + j * 128 : r16 * 256 + j * 128 + 128],
                    wt[:, (r16 % 4) * 512 : (r16 % 4 + 1) * 512],
                    start=(r16 == 0),
                    stop=(r16 == 2 * NH - 1),
                )
            osb = p_osb.tile([128, 512], BF16, tag="osb")
            nc.vector.tensor_copy(osb[:, :], ps_out[:, :])
            nc.sync.dma_start(
                out=out[2 * c + j, :, do * 512 : (do + 1) * 512],
                in_=osb[:, :],
            )
            if t == 7:
                ostg.pop(c, None)

        # Q(0) stands alone; thereafter Q(n+1) head f interleaves with
        # attention chunk n head f, so independent projection matmuls fill
        # the PE queue wherever attention stalls on the exp pipeline.
        xtq = xt_pending.pop(0)
        for f in range(NH):
            q_head(0, f, xtq)
        prev1 = prev2 = None  # prev1 awaits fin_a, prev2 awaits fin_b
        for n in range(NQ):
            xt_prev, xtq = xtq, None
            if n + 1 < NQ:
                xtq = load_xt_quarter(n + 1, [nc.sync])
            if n == 1:
                issue_woth(2)
            for eta in range(NH):
                def fin(p1=prev1, p2=prev2):
                    if p1 is not None:
                        fin_a(p1)
                    if p2 is not None:
                        fin_b(p2)
                prev2, prev1 = prev1, attn_head(n, eta, fin)
                if n + 1 < NQ:
                    q_head(n + 1, eta, xtq)
                # o-proj tiles, three head-steps behind the chunk's AGs so
                # every peer gather lands at least one step before its
                # first consumer
                if n == 1 and eta == 1:
                    issue_woth(2)
                if eta >= 3 and n >= 1:
                    oproj_tile(n - 1, eta - 3)
                elif eta < 3 and n >= 2:
                    oproj_tile(n - 2, 5 + eta)
        # tail: flush fins with no gather drains between the final AllToAll
        # triggers, fill the collective flight time with chunk-2 tiles,
        # then drain and run chunk 3.  woth loads move to the sync ring so
        # they never queue behind AG-waiting gathers on gpsimd.
        woth_eng[0] = nc.sync
        fin_a(prev1)
        fin_b(prev2, drain=False)
        fin_b(prev1, drain=False)
        for t in (5, 6, 7):
            oproj_tile(2, t)
        drain_peer_gathers(0)
        for t in range(8):
            oproj_tile(3, t)

        for name in reversed(list(_cm)):
            pclose(name)

    nc.finalize()
    return nc


def host_prep(T, D, H, x, wq, wv, wo, mask, freqs_cos, freqs_sin):
    """Build per-core in_maps (host-side layout/dtype prep only)."""
    import ml_dtypes

    bf16 = ml_dtypes.bfloat16
    HD = 128
    NH = H // 2
    DQ = NH * HD
    # 0/1 keep-mask (from the additive mask) for post-exp diagonal zeroing
    m128 = np.asarray(mask, np.float32).reshape(T, T)[:128, :128]
    mkt = np.ascontiguousarray(np.where(m128 < -1e8, 0.0, 1.0).astype(bf16))
    fcn = np.asarray(freqs_cos, np.float32)  # [T, 64]
    fsn = np.asarray(freqs_sin, np.float32)
    c2t = fcn.T * fcn.T - fsn.T * fsn.T   # cos(2a)  [64, T]
    s2t = 2.0 * fcn.T * fsn.T             # sin(2a)
    # partitions 0:64 cos2, 64:128 sin2
    fcs = np.ascontiguousarray(np.concatenate([c2t, s2t], axis=0).astype(np.float32))
    # deinterleave permutation: within each head block, (r0,r1,..,i0,i1,..)
    perm = np.concatenate(
        [hb * 128 + np.r_[0:128:2, 1:128:2] for hb in range(NH)]
    )
    wot_full = np.asarray(wo, np.float32).T  # [din2, dout]
    in_maps = []
    for c in range(N_CORES):
        b, h = c // 2, c % 2
        rows = slice(h * DQ, (h + 1) * DQ)
        wqt_c = np.asarray(wq[rows], np.float32).T[:, perm]
        # stg rows are own-heads-first: permute wot accordingly
        wot_c = np.concatenate(
            [wot_full[h * DQ : (h + 1) * DQ], wot_full[(1 - h) * DQ : (2 - h) * DQ]],
            axis=0,
        )
        # tile to SBUF images: [dk-major columns, partition-major rows]
        def timg(a, ncol):  # [D, C] -> [C//ncol, 128, 16*ncol]
            ND_, C = a.shape[0] // 128, a.shape[1]
            return np.ascontiguousarray(
                a.reshape(ND_, 128, C // ncol, ncol)
                .transpose(2, 1, 0, 3)
                .reshape(C // ncol, 128, ND_ * ncol)
                .astype(bf16)
            )
        # xt: j-major within each quarter: [n, p, j, dk, t]
        xT = np.asarray(x[b], np.float32).T                      # [D, T]
        xtc = np.ascontiguousarray(
            xT.reshape(16, 128, 4, 4, 128)                       # dk,p,n,j,t
            .transpose(2, 1, 3, 0, 4)
            .reshape(4, 128, 4 * 16 * 128)
            .astype(bf16)
        )
        wqtc = timg(wqt_c, DQ).reshape(128, -1)                  # [128,16384]
        # wvt: qc-major: [p, qc, dk, c]
        wvT = np.asarray(wv[rows], np.float32).T                 # [D, DQ]
        wvtc = np.ascontiguousarray(
            wvT.reshape(16, 128, 2, 512)                         # dk,p,qc,c
            .transpose(1, 2, 0, 3)
            .reshape(128, 2 * 16 * 512)
            .astype(bf16)
        )
        # woth: [do, quarter, p, r16local, c]; rows own-heads-first
        wothc = np.ascontiguousarray(
            wot_c.reshape(4, 4, 128, 4, 512)     # q, r16l, p, do, c
            .transpose(3, 0, 2, 1, 4)
            .reshape(4, 4, 128, 4 * 512)
            .astype(bf16)
        )
        in_maps.append(
            {
                "xt": xtc,
                "wqt": wqtc,
                "wvt": wvtc,
                "woth": wothc,
                "maskt": mkt,
                "fcs": fcs,
            }
        )
    return in_maps


_NC_CACHE = {}


def run(T, D, H, inputs, trace=False):
    from concourse.bass_utils import run_bass_kernel_spmd

    key = (T, D, H)
    if key not in _NC_CACHE:
        _NC_CACHE[key] = build_nc(T, D, H)
    nc = _NC_CACHE[key]
    in_maps = host_prep(
        T, D, H,
        inputs["x"], inputs["wq"], inputs["wv"], inputs["wo"],
        inputs["mask"], inputs["freqs_cos"], inputs["freqs_sin"],
    )
    res = run_bass_kernel_spmd(nc, in_maps, core_ids=list(range(N_CORES)), trace=trace)
    B_ = np.asarray(inputs["x"]).shape[0]
    out = np.empty((B_, T, D), np.float32)
    for c in range(N_CORES):
        b, h = c // 2, c % 2
        oc = np.asarray(res.results[c]["out"], np.float32)  # [8,128,D]
        for ck in range(4):
            for j in range(2):
                t0 = (4 * ck + 2 * h + j) * 128
                out[b, t0 : t0 + 128, :] = oc[2 * ck + j]
    return out, res


def kernel(**inputs):
    out, _ = run(T, D, H, inputs, trace=False)
    return out

